# revision 11
# baseline (speedup 1.0000x reference)
"""Trainium2 Bass kernel for nn_PositionalAttentionHead.

Problem: per-head LayerNorm -> shared QKV projections (64x64) -> RoPE ->
attention with safeguarded softmax.  Returns (out [4,1024,1024],
probs [4,16,1024,1024]).

Sharding: the 64 (batch, head) units are fully independent; 8 units per
NeuronCore, no collectives.  Each core's 8 units share one batch index so
the attention-mask bias row is a single per-core constant.

Per-unit device pipeline (S=1024, D=64):
  P0  load x slice [128, 8, 64]; LayerNorm stats via bn_stats/bn_aggr;
      rstd = exp(-0.5*ln(var+eps)) (keeps ACT on one table set: exp+ln);
      z = (x-mu)*rstd with ones column; PE-transpose to zT_aug [65, 1024].
  P1  projections: gamma/beta folded into weights host-side (K=65
      augmentation adds the beta-bias); RoPE via a second projection with
      the rotate_half permutation folded into the weights, then
      q' = cos*q_raw + sin*q_rot on DVE/gpsimd.  k'T_aug row 64 carries
      the mask bias, q'T_aug row 64 carries ones.
  P2  scoresT [k,q] tiles on PE (mask bias added by the K=65 term);
      eT = exp(scoresT/8) on ACT; out accumulation with v augmented by a
      ones column so column 64 of outT accumulates the softmax row-sum r.
  P3  PE-transpose outT -> [q, 65]; recip = 1/(r+1e-7);
      out = outT[:, :64] * recip; negL = ln(recip).
  P4  scores [q,k] tiles on PE; probs = exp(scores/8 + negL) on ACT
      (exactly e/(r+1e-7), the reference safeguard with m=0 -- the max
      subtraction only perturbs the 1e-7 safeguard, rel err <= 1e-7).
"""

import os
from contextlib import ExitStack

import numpy as np

import concourse.bacc as bacc
import concourse.mybir as mybir
import concourse.tile as tile
from concourse.bass_utils import run_bass_kernel_spmd

F32 = mybir.dt.float32
F32R = mybir.dt.float32r

B, S, HID, NH, HD = 4, 1024, 1024, 16, 64
NCORES = 8
UPC = 8  # units per core; unit u (global) = (b, h) with b = u // NH
NT = S // 128  # 8 q/k chunks of 128
LN_EPS = 1e-5
MASK_NEG = -8.0e30  # pre-scale bias; exp(0.125 * -8e30) == 0

# Matmul precision: F32 is exact (4 cycles/row on PE); F32R is the fast
# fp32 mode (1 cycle/row at N>=512) with ~2e-4 operand rounding.
MM_DTYPE = F32R if os.environ.get("KERNEL_F32R", "1") == "1" else F32

# consts block layout (free offsets, f32, [128, CW])
OFF_COS = 0          # [0:64,    0:1024]  cos table, [d, s]
OFF_SIN = 1024       # [0:64, 1024:2048]  sin table (rows 0:64)
OFF_KBIAS = 1024     # [64:65,1024:2048]  mask bias row (partition 64!)
OFF_ID = 2048        # [0:128,2048:2176]  identity
OFF_W = 2176         # [0:65, 2176:2496]  Wq, Wq_rot, Wk, Wk_rot, Wv
OFF_ONES = 2496      # [0:128,2496:3520]  all ones
CW = 3584

ACT = mybir.ActivationFunctionType
ALU = mybir.AluOpType

_CACHE = {}


def _build_program():
    nc = bacc.Bacc("TRN2", target_bir_lowering=False)
    xs = nc.declare_dram_parameter("xs", [UPC, 128, 8 * HD], F32,
                                   isOutput=False)
    consts = nc.declare_dram_parameter("consts", [128, CW], F32,
                                       isOutput=False)
    probs_o = nc.declare_dram_parameter("probs_o", [UPC, S, S], F32,
                                        isOutput=True)
    out_o = nc.declare_dram_parameter("out_o", [UPC, S, HD], F32,
                                      isOutput=True)

    with tile.TileContext(nc) as tc, ExitStack() as ctx:
        cpool = ctx.enter_context(tc.tile_pool(name="cpool", bufs=1))
        sb = ctx.enter_context(tc.tile_pool(name="sb", bufs=2))
        sb3 = ctx.enter_context(tc.tile_pool(name="sb3", bufs=3))
        ps_big = ctx.enter_context(
            tc.tile_pool(name="ps_big", bufs=2, space="PSUM"))
        ps_proj = ctx.enter_context(
            tc.tile_pool(name="ps_proj", bufs=3, space="PSUM"))
        ps_small = ctx.enter_context(
            tc.tile_pool(name="ps_small", bufs=1, space="PSUM"))
        ps_outt = ctx.enter_context(
            tc.tile_pool(name="ps_outt", bufs=1, space="PSUM"))

        ct = cpool.tile([128, CW], F32, tag="ct")
        nc.gpsimd.dma_start(out=ct, in_=consts[:])
        cosT = ct[0:64, OFF_COS:OFF_COS + S]
        sinT = ct[0:64, OFF_SIN:OFF_SIN + S]
        kbias_row = ct[64:65, OFF_KBIAS:OFF_KBIAS + S]
        ident = ct[:, OFF_ID:OFF_ID + 128]
        ident65 = ct[0:65, OFF_ID:OFF_ID + 65]

        # weights cast to the matmul dtype once (on-chip produce rule
        # for f32r)
        wcast = cpool.tile([65, 5 * HD], MM_DTYPE, tag="wcast")
        nc.vector.tensor_copy(wcast, ct[0:65, OFF_W:OFF_W + 5 * HD])

        zero_col = cpool.tile([128, 1], F32, tag="zero_col")
        nc.vector.memset(zero_col, 0.0)
        eps_col = cpool.tile([128, 1], F32, tag="eps_col")
        nc.vector.memset(eps_col, LN_EPS)
        w_q = wcast[:, 0 * HD:1 * HD]
        w_qr = wcast[:, 1 * HD:2 * HD]
        w_k = wcast[:, 2 * HD:3 * HD]
        w_kr = wcast[:, 3 * HD:4 * HD]
        w_v = wcast[:, 4 * HD:5 * HD]

        for u in range(UPC):
            # ---- P0: load + LayerNorm + transpose -------------------
            xt = sb.tile([128, NT, HD], F32, tag="xt")
            nc.gpsimd.dma_start(
                out=xt, in_=xs[u].rearrange("p (t d) -> p t d", d=HD))

            stats = sb.tile([128, NT, 6], F32, tag="stats")
            mv = sb.tile([128, NT, 2], F32, tag="mv")
            for t in range(NT):
                nc.vector.bn_stats(stats[:, t, :], xt[:, t, :])
                nc.vector.bn_aggr(mv[:, t, :], stats[:, t, :])
            lnv = sb.tile([128, NT], F32, tag="lnv")
            nc.scalar.activation(lnv, mv[:, :, 1], ACT.Ln, bias=eps_col)
            rstd = sb.tile([128, NT], F32, tag="rstd")
            nc.scalar.activation(rstd, lnv, ACT.Exp, scale=-0.5,
                                 bias=zero_col)

            z = sb.tile([128, NT, HD + 1], F32, tag="z")
            nc.vector.memset(z[:, :, HD:HD + 1], 1.0)
            for t in range(NT):
                nc.vector.tensor_scalar(
                    z[:, t, 0:HD], xt[:, t, :], mv[:, t, 0:1],
                    rstd[:, t:t + 1], ALU.subtract, ALU.mult)

            zT = sb.tile([65, S], MM_DTYPE, tag="zT")
            for half in range(2):
                pzt = ps_big.tile([128, 512], F32, tag="big")
                for t4 in range(4):
                    t = half * 4 + t4
                    nc.tensor.transpose(
                        pzt[0:65, t4 * 128:(t4 + 1) * 128], z[:, t, :],
                        ident)
                nc.vector.tensor_copy(
                    zT[:, half * 512:(half + 1) * 512], pzt[0:65, :])

            # ---- P1: projections + rope -----------------------------
            qT = sb.tile([65, S], MM_DTYPE, tag="qT")
            kT = sb.tile([65, S], MM_DTYPE, tag="kT")
            nc.vector.tensor_copy(qT[64:65, :],
                                  ct[64:65, OFF_ONES:OFF_ONES + S])
            nc.vector.tensor_copy(kT[64:65, :], kbias_row)

            for dst, w_s, w_r in ((qT, w_q, w_qr), (kT, w_k, w_kr)):
                for c in range(2):
                    zc = zT[:, c * 512:(c + 1) * 512]
                    praw = ps_proj.tile([128, 512], F32, tag="proj")
                    prot = ps_proj.tile([128, 512], F32, tag="proj")
                    nc.tensor.matmul(praw[0:64, :], w_s, zc,
                                     start=True, stop=True)
                    nc.tensor.matmul(prot[0:64, :], w_r, zc,
                                     start=True, stop=True)
                    m1 = sb.tile([64, 512], F32, tag="m1")
                    m2 = sb.tile([64, 512], F32, tag="m2")
                    nc.vector.tensor_mul(
                        m1, praw[0:64, :], cosT[:, c * 512:(c + 1) * 512])
                    nc.vector.tensor_mul(
                        m2, prot[0:64, :], sinT[:, c * 512:(c + 1) * 512])
                    nc.gpsimd.tensor_add(
                        dst[0:64, c * 512:(c + 1) * 512], m1, m2)

            vaug = sb.tile([128, NT, 65], MM_DTYPE, tag="vaug")
            nc.vector.tensor_copy(
                vaug[:, :, HD:HD + 1],
                ct[:, OFF_ONES:OFF_ONES + NT].unsqueeze(2))
            for t in range(NT):
                pv = ps_proj.tile([128, 512], F32, tag="proj")
                nc.tensor.matmul(
                    pv[:, 0:HD], zT[:, t * 128:(t + 1) * 128], w_v,
                    start=True, stop=True)
                nc.scalar.copy(vaug[:, t, 0:HD], pv[:, 0:HD])

            # ---- P2: scoresT -> eT -> out accumulation --------------
            poutT = ps_outt.tile([65, S], F32, tag="outT")
            for kc in range(NT):
                for h in range(2):
                    pst = ps_big.tile([128, 512], F32, tag="big")
                    nc.tensor.matmul(
                        pst, kT[:, kc * 128:(kc + 1) * 128],
                        qT[:, h * 512:(h + 1) * 512],
                        start=True, stop=True)
                    eT = sb3.tile([128, 512], MM_DTYPE, tag="eT")
                    nc.scalar.activation(eT, pst, ACT.Exp, scale=0.125,
                                         bias=zero_col)
                    nc.tensor.matmul(
                        poutT[:, h * 512:(h + 1) * 512], vaug[:, kc, :],
                        eT, start=(kc == 0), stop=(kc == NT - 1),
                        skip_group_check=True)

            # ---- P3: finalize out, per q-chunk ----------------------
            outT_sb = sb.tile([65, S], F32, tag="outTsb")
            nc.scalar.copy(outT_sb, poutT)
            outsb = sb.tile([128, NT, HD], F32, tag="outsb")
            negL = sb.tile([128, NT], F32, tag="negL")
            recips = sb.tile([128, NT], F32, tag="recips")
            rplus = sb.tile([128, NT], F32, tag="rplus")
            for t in range(NT):
                pot = ps_small.tile([128, 65], F32, tag="small")
                nc.tensor.transpose(
                    pot, outT_sb[:, t * 128:(t + 1) * 128], ident65)
                nc.vector.tensor_scalar_add(
                    rplus[:, t:t + 1], pot[:, 64:65], 1e-7)
                nc.vector.reciprocal(recips[:, t:t + 1], rplus[:, t:t + 1])
                nc.vector.tensor_scalar_mul(
                    outsb[:, t, :], pot[:, 0:HD], recips[:, t:t + 1])
                nc.scalar.activation(
                    negL[:, t:t + 1], recips[:, t:t + 1], ACT.Ln,
                    bias=zero_col)
            nc.sync.dma_start(
                out=out_o[u].rearrange("(t p) d -> p t d", p=128),
                in_=outsb)

            # ---- P4: scores -> probs --------------------------------
            for t in range(NT):
                prb = sb3.tile([128, S], F32, tag="prb")
                for h in range(2):
                    psc = ps_big.tile([128, 512], F32, tag="big")
                    nc.tensor.matmul(
                        psc, qT[:, t * 128:(t + 1) * 128],
                        kT[:, h * 512:(h + 1) * 512],
                        start=True, stop=True)
                    nc.scalar.activation(
                        prb[:, h * 512:(h + 1) * 512], psc, ACT.Exp,
                        scale=0.125, bias=negL[:, t:t + 1])
                nc.sync.dma_start(
                    out=probs_o[u, t * 128:(t + 1) * 128, :], in_=prb)

    nc.compile()
    return nc


def _rope_tables():
    inv_freq = 1.0 / (10000.0 ** (np.arange(0, HD, 2, dtype=np.float64)
                                  / HD))
    t = np.arange(S, dtype=np.float64)
    freqs = t[:, None] * inv_freq[None, :]          # [S, 32]
    emb = np.concatenate((freqs, freqs), axis=-1)   # [S, 64]
    return (np.cos(emb).astype(np.float32).T.copy(),
            np.sin(emb).astype(np.float32).T.copy())  # [64, S]


def _rot_cols(w_full):
    """Fold rotate_half into the stationary weights.

    w_full is [65, 64] with q = zT_aug.T @ w_full.  Returns w_rot such
    that zT_aug.T @ w_rot = rotate_half(q):
    rot(q)[e] = -q[e+32] for e<32, q[e-32] for e>=32.
    """
    w_rot = np.empty_like(w_full)
    w_rot[:, 0:32] = -w_full[:, 32:64]
    w_rot[:, 32:64] = w_full[:, 0:32]
    return w_rot


def kernel(x, attention_mask, ln_gamma, ln_beta, Wq, Wk, Wv):
    x = np.asarray(x, dtype=np.float32)
    attention_mask = np.asarray(attention_mask).astype(bool)
    ln_gamma = np.asarray(ln_gamma, dtype=np.float32)
    ln_beta = np.asarray(ln_beta, dtype=np.float32)
    Wq = np.asarray(Wq, dtype=np.float32)
    Wk = np.asarray(Wk, dtype=np.float32)
    Wv = np.asarray(Wv, dtype=np.float32)

    if "nc" not in _CACHE:
        _CACHE["nc"] = _build_program()
    nc = _CACHE["nc"]

    # x slices per unit: [64, 128, 512] with [u, p, t*64+d] = x[b, t*128+p,
    # h*64+d]
    xh = x.reshape(B, S, NH, HD).transpose(0, 2, 1, 3)  # [b, h, s, d]
    xs_all = (xh.reshape(B * NH, NT, 128, HD)
                .transpose(0, 2, 1, 3)
                .reshape(B * NH, 128, NT * HD)
                .astype(np.float32))

    cosT, sinT = _rope_tables()

    def w_full(W):
        ws = (W.T * ln_gamma[:, None]).astype(np.float32)   # [d, e]
        bias = (W @ ln_beta).astype(np.float32)             # [e]
        return np.concatenate([ws, bias[None, :]], axis=0)  # [65, e]

    wq_f = w_full(Wq)
    wk_f = w_full(Wk)
    wv_f = w_full(Wv)

    consts_base = np.zeros((128, CW), dtype=np.float32)
    consts_base[0:64, OFF_COS:OFF_COS + S] = cosT
    consts_base[0:64, OFF_SIN:OFF_SIN + S] = sinT
    consts_base[:, OFF_ID:OFF_ID + 128] = np.eye(128, dtype=np.float32)
    consts_base[:, OFF_ONES:OFF_ONES + S] = 1.0
    wblock = np.concatenate(
        [wq_f, _rot_cols(wq_f), wk_f, _rot_cols(wk_f), wv_f], axis=1)
    consts_base[0:65, OFF_W:OFF_W + 5 * HD] = wblock

    kbias = np.where(attention_mask, 0.0, MASK_NEG).astype(np.float32)

    in_maps = []
    for c in range(NCORES):
        consts_c = consts_base.copy()
        b = (c * UPC) // NH
        consts_c[64, OFF_KBIAS:OFF_KBIAS + S] = kbias[b]
        in_maps.append({
            "xs": xs_all[c * UPC:(c + 1) * UPC],
            "consts": consts_c,
        })

    res = run_bass_kernel_spmd(nc, in_maps, list(range(NCORES)))
    _CACHE["last_result"] = res

    probs = np.concatenate(
        [res.results[c]["probs_o"] for c in range(NCORES)], axis=0)
    probs = probs.reshape(B, NH, S, S)
    out = np.concatenate(
        [res.results[c]["out_o"] for c in range(NCORES)], axis=0)
    out = (out.reshape(B, NH, S, HD)
              .transpose(0, 2, 1, 3)
              .reshape(B, S, HID)
              .astype(np.float32))
    return out, probs


# revision 22
# speedup vs baseline: 1.9559x; 1.9559x over previous
"""Trainium2 Bass kernel for nn_PositionalAttentionHead.

Problem: per-head LayerNorm -> shared QKV projections (64x64) -> RoPE ->
attention with safeguarded softmax.  Returns (out [4,1024,1024],
probs [4,16,1024,1024]).

Sharding: the 64 (batch, head) units are fully independent; 8 units per
NeuronCore, no collectives.  Each core's 8 units share one batch index so
the attention-mask bias row is a single per-core constant.

Per-unit device pipeline (S=1024, D=64):
  P0  load x slice [128, 8, 64]; LayerNorm stats via bn_stats/bn_aggr;
      rstd = exp(-0.5*ln(var+eps)) (keeps ACT on one table set: exp+ln);
      z = (x-mu)*rstd with ones column; PE-transpose to zT_aug [65, 1024].
  P1  projections: gamma/beta folded into weights host-side (K=65
      augmentation adds the beta-bias); RoPE via a second projection with
      the rotate_half permutation folded into the weights, then
      q' = cos*q_raw + sin*q_rot on DVE/gpsimd.  k'T_aug row 64 carries
      the mask bias, q'T_aug row 64 carries ones.
  P2  scoresT [k,q] tiles on PE (mask bias added by the K=65 term);
      eT = exp(scoresT/8) on ACT; out accumulation with v augmented by a
      ones column so column 64 of outT accumulates the softmax row-sum r.
  P3  PE-transpose outT -> [q, 65]; recip = 1/(r+1e-7);
      out = outT[:, :64] * recip; negL = ln(recip).
  P4  scores [q,k] tiles on PE; probs = exp(scores/8 + negL) on ACT
      (exactly e/(r+1e-7), the reference safeguard with m=0 -- the max
      subtraction only perturbs the 1e-7 safeguard, rel err <= 1e-7).
"""

import os
from contextlib import ExitStack

import numpy as np

import concourse.bacc as bacc
import concourse.mybir as mybir
import concourse.tile as tile
from concourse.bass_utils import run_bass_kernel_spmd

F32 = mybir.dt.float32
F32R = mybir.dt.float32r

B, S, HID, NH, HD = 4, 1024, 1024, 16, 64
NCORES = 8
UPC = 8  # units per core; unit u (global) = (b, h) with b = u // NH
NT = S // 128  # 8 q/k chunks of 128
LN_EPS = 1e-5
MASK_NEG = -8.0e30  # pre-scale bias; exp(0.125 * -8e30) == 0

# Matmul precision: F32 is exact (4 cycles/row on PE); F32R is the fast
# fp32 mode (1 cycle/row at N>=512) with ~2e-4 operand rounding.
MM_DTYPE = F32R if os.environ.get("KERNEL_F32R", "1") == "1" else F32

# consts block layout (free offsets, f32, [128, CW])
OFF_COS = 0          # [0:64,    0:1024]  cos table, [d, s]
OFF_SIN = 1024       # [0:64, 1024:2048]  sin table (rows 0:64)
OFF_KBIAS = 1024     # [64:65,1024:2048]  mask bias row (partition 64!)
OFF_ID = 2048        # [0:128,2048:2176]  identity
OFF_W = 2176         # [0:65, 2176:2496]  Wq, Wq_rot, Wk, Wk_rot, Wv
OFF_ONES = 2496      # [0:128,2496:3520]  all ones
CW = 3584

ACT = mybir.ActivationFunctionType
ALU = mybir.AluOpType

_CACHE = {}


def _patched_act_tables(arch):
    """Keep Exp/Ln only in natural_log_exp_and_others so the act-table
    chooser picks that one set for every activation (1 table load instead
    of ~112 Exp<->Ln thrash swaps at ~2.7us each).  Dict order (the
    act_func_set_id indices) is preserved."""
    from concourse.hw_specs import get_activation_tables
    tabs = dict(get_activation_tables(arch))
    keep = "natural_log_exp_and_others"
    out = {}
    for name, funcs in tabs.items():
        if name == keep:
            out[name] = funcs
        else:
            out[name] = {f for f in funcs
                         if f not in (ACT.Exp, ACT.Ln)}
    return out


def _build_program():
    nc = bacc.Bacc("TRN2", target_bir_lowering=False)
    xs = nc.declare_dram_parameter("xs", [UPC, 128, 8 * HD], F32,
                                   isOutput=False)
    consts = nc.declare_dram_parameter("consts", [128, CW], F32,
                                       isOutput=False)
    probs_o = nc.declare_dram_parameter("probs_o", [UPC, S, S], F32,
                                        isOutput=True)
    out_o = nc.declare_dram_parameter("out_o", [UPC, S, HD], F32,
                                      isOutput=True)

    with tile.TileContext(nc) as tc, ExitStack() as ctx:
        cpool = ctx.enter_context(tc.tile_pool(name="cpool", bufs=1))
        sb = ctx.enter_context(tc.tile_pool(name="sb", bufs=2))
        sb3 = ctx.enter_context(tc.tile_pool(name="sb3", bufs=3))
        ps_st = ctx.enter_context(
            tc.tile_pool(name="ps_st", bufs=2, space="PSUM"))
        ps_s = ctx.enter_context(
            tc.tile_pool(name="ps_s", bufs=2, space="PSUM"))
        ps_proj = ctx.enter_context(
            tc.tile_pool(name="ps_proj", bufs=2, space="PSUM"))
        ps_outt = ctx.enter_context(
            tc.tile_pool(name="ps_outt", bufs=1, space="PSUM"))

        ct = cpool.tile([128, CW], F32, tag="ct")
        nc.gpsimd.dma_start(out=ct, in_=consts[:])
        cosT = ct[0:64, OFF_COS:OFF_COS + S]
        sinT = ct[0:64, OFF_SIN:OFF_SIN + S]
        kbias_row = ct[64:65, OFF_KBIAS:OFF_KBIAS + S]
        ident = ct[:, OFF_ID:OFF_ID + 128]
        ident65 = ct[0:65, OFF_ID:OFF_ID + 65]

        # weights cast to the matmul dtype once (on-chip produce rule
        # for f32r)
        wcast = cpool.tile([65, 5 * HD], MM_DTYPE, tag="wcast")
        nc.vector.tensor_copy(wcast, ct[0:65, OFF_W:OFF_W + 5 * HD])

        zero_col = cpool.tile([128, 1], F32, tag="zero_col")
        nc.vector.memset(zero_col, 0.0)
        eps_col = cpool.tile([128, 1], F32, tag="eps_col")
        nc.vector.memset(eps_col, LN_EPS)
        w_q = wcast[:, 0 * HD:1 * HD]
        w_qr = wcast[:, 1 * HD:2 * HD]
        w_k = wcast[:, 2 * HD:3 * HD]
        w_kr = wcast[:, 3 * HD:4 * HD]
        w_v = wcast[:, 4 * HD:5 * HD]

        for u in range(UPC):
            # ---- P0: load + LayerNorm + transpose -------------------
            xt = sb.tile([128, NT, HD], F32, tag="xt")
            nc.gpsimd.dma_start(
                out=xt, in_=xs[u].rearrange("p (t d) -> p t d", d=HD))

            stats = sb.tile([128, NT, 6], F32, tag="stats")
            mv = sb.tile([128, NT, 2], F32, tag="mv")
            for t in range(NT):
                nc.vector.bn_stats(stats[:, t, :], xt[:, t, :])
                nc.vector.bn_aggr(mv[:, t, :], stats[:, t, :])
            lnv = sb.tile([128, NT], F32, tag="lnv")
            nc.scalar.activation(lnv, mv[:, :, 1], ACT.Ln, bias=eps_col)
            rstd = sb.tile([128, NT], F32, tag="rstd")
            nc.scalar.activation(rstd, lnv, ACT.Exp, scale=-0.5,
                                 bias=zero_col)

            z = sb.tile([128, NT, HD + 1], F32, tag="z")
            nc.vector.memset(z[:, :, HD:HD + 1], 1.0)
            for t in range(NT):
                nc.vector.tensor_scalar(
                    z[:, t, 0:HD], xt[:, t, :], mv[:, t, 0:1],
                    rstd[:, t:t + 1], ALU.subtract, ALU.mult)

            zT = sb.tile([65, S], MM_DTYPE, tag="zT")
            for half in range(2):
                pzt = ps_proj.tile([128, 512], F32, tag="proj")
                for t4 in range(4):
                    t = half * 4 + t4
                    nc.tensor.transpose(
                        pzt[0:65, t4 * 128:(t4 + 1) * 128], z[:, t, :],
                        ident)
                nc.vector.tensor_copy(
                    zT[:, half * 512:(half + 1) * 512], pzt[0:65, :])

            # ---- P1: projections + rope -----------------------------
            qT = sb.tile([65, S], MM_DTYPE, tag="qT")
            kT = sb.tile([65, S], MM_DTYPE, tag="kT")
            nc.vector.tensor_copy(qT[64:65, :],
                                  ct[64:65, OFF_ONES:OFF_ONES + S])
            nc.vector.tensor_copy(kT[64:65, :], kbias_row)

            for dst, w_s, w_r in ((qT, w_q, w_qr), (kT, w_k, w_kr)):
                for c in range(2):
                    zc = zT[:, c * 512:(c + 1) * 512]
                    praw = ps_proj.tile([128, 512], F32, tag="proj")
                    prot = ps_proj.tile([128, 512], F32, tag="proj")
                    nc.tensor.matmul(praw[0:64, :], w_s, zc,
                                     start=True, stop=True)
                    nc.tensor.matmul(prot[0:64, :], w_r, zc,
                                     start=True, stop=True)
                    m1 = sb.tile([64, 512], F32, tag="m1")
                    m2 = sb.tile([64, 512], F32, tag="m2")
                    nc.vector.tensor_mul(
                        m1, praw[0:64, :], cosT[:, c * 512:(c + 1) * 512])
                    nc.vector.tensor_mul(
                        m2, prot[0:64, :], sinT[:, c * 512:(c + 1) * 512])
                    nc.gpsimd.tensor_add(
                        dst[0:64, c * 512:(c + 1) * 512], m1, m2)

            vaug = sb.tile([128, NT, 65], MM_DTYPE, tag="vaug")
            nc.vector.tensor_copy(
                vaug[:, :, HD:HD + 1],
                ct[:, OFF_ONES:OFF_ONES + NT].unsqueeze(2))
            for t in range(NT):
                pv = ps_proj.tile([128, 512], F32, tag="proj")
                nc.tensor.matmul(
                    pv[:, 0:HD], zT[:, t * 128:(t + 1) * 128], w_v,
                    start=True, stop=True)
                nc.vector.tensor_copy(vaug[:, t, 0:HD], pv[:, 0:HD])

            # ---- P2: scoresT -> eT -> out accumulation --------------
            poutT = ps_outt.tile([65, S], F32, tag="outT")
            for kc in range(NT):
                for h in range(2):
                    pst = ps_st.tile([128, 512], F32, tag="st")
                    nc.tensor.matmul(
                        pst, kT[:, kc * 128:(kc + 1) * 128],
                        qT[:, h * 512:(h + 1) * 512],
                        start=True, stop=True)
                    eT = sb3.tile([128, 512], MM_DTYPE, tag="eT")
                    nc.scalar.activation(eT, pst, ACT.Exp, scale=0.125,
                                         bias=zero_col)
                    nc.tensor.matmul(
                        poutT[:, h * 512:(h + 1) * 512], vaug[:, kc, :],
                        eT, start=(kc == 0), stop=(kc == NT - 1),
                        skip_group_check=True)

            # ---- P3: finalize out, per q-chunk ----------------------
            outT_sb = sb.tile([65, S], F32, tag="outTsb")
            nc.vector.tensor_copy(outT_sb, poutT)
            outsb = sb.tile([128, NT, HD], F32, tag="outsb")
            negL = sb.tile([128, NT], F32, tag="negL")
            recips = sb.tile([128, NT], F32, tag="recips")
            rplus = sb.tile([128, NT], F32, tag="rplus")
            for t in range(NT):
                pot = ps_s.tile([128, 512], F32, tag="s")
                nc.tensor.transpose(
                    pot[:, 0:65], outT_sb[:, t * 128:(t + 1) * 128],
                    ident65)
                nc.vector.tensor_scalar_add(
                    rplus[:, t:t + 1], pot[:, 64:65], 1e-7)
                nc.vector.reciprocal(recips[:, t:t + 1], rplus[:, t:t + 1])
                nc.vector.tensor_scalar_mul(
                    outsb[:, t, :], pot[:, 0:HD], recips[:, t:t + 1])
                nc.scalar.activation(
                    negL[:, t:t + 1], recips[:, t:t + 1], ACT.Ln,
                    bias=zero_col)
            nc.sync.dma_start(
                out=out_o[u].rearrange("(t p) d -> p t d", p=128),
                in_=outsb)

            # ---- P4: scores -> probs --------------------------------
            for t in range(NT):
                prb = sb3.tile([128, S], F32, tag="prb")
                for h in range(2):
                    psc = ps_s.tile([128, 512], F32, tag="s")
                    nc.tensor.matmul(
                        psc, qT[:, t * 128:(t + 1) * 128],
                        kT[:, h * 512:(h + 1) * 512],
                        start=True, stop=True)
                    nc.scalar.activation(
                        prb[:, h * 512:(h + 1) * 512], psc, ACT.Exp,
                        scale=0.125, bias=negL[:, t:t + 1])
                nc.sync.dma_start(
                    out=probs_o[u, t * 128:(t + 1) * 128, :], in_=prb)

    _orig = bacc.get_activation_tables
    bacc.get_activation_tables = _patched_act_tables
    try:
        nc.compile()
    finally:
        bacc.get_activation_tables = _orig
    return nc


def _rope_tables():
    inv_freq = 1.0 / (10000.0 ** (np.arange(0, HD, 2, dtype=np.float64)
                                  / HD))
    t = np.arange(S, dtype=np.float64)
    freqs = t[:, None] * inv_freq[None, :]          # [S, 32]
    emb = np.concatenate((freqs, freqs), axis=-1)   # [S, 64]
    return (np.cos(emb).astype(np.float32).T.copy(),
            np.sin(emb).astype(np.float32).T.copy())  # [64, S]


def _rot_cols(w_full):
    """Fold rotate_half into the stationary weights.

    w_full is [65, 64] with q = zT_aug.T @ w_full.  Returns w_rot such
    that zT_aug.T @ w_rot = rotate_half(q):
    rot(q)[e] = -q[e+32] for e<32, q[e-32] for e>=32.
    """
    w_rot = np.empty_like(w_full)
    w_rot[:, 0:32] = -w_full[:, 32:64]
    w_rot[:, 32:64] = w_full[:, 0:32]
    return w_rot


def kernel(x, attention_mask, ln_gamma, ln_beta, Wq, Wk, Wv):
    x = np.asarray(x, dtype=np.float32)
    attention_mask = np.asarray(attention_mask).astype(bool)
    ln_gamma = np.asarray(ln_gamma, dtype=np.float32)
    ln_beta = np.asarray(ln_beta, dtype=np.float32)
    Wq = np.asarray(Wq, dtype=np.float32)
    Wk = np.asarray(Wk, dtype=np.float32)
    Wv = np.asarray(Wv, dtype=np.float32)

    if "nc" not in _CACHE:
        _CACHE["nc"] = _build_program()
    nc = _CACHE["nc"]

    # x slices per unit: [64, 128, 512] with [u, p, t*64+d] = x[b, t*128+p,
    # h*64+d]
    xh = x.reshape(B, S, NH, HD).transpose(0, 2, 1, 3)  # [b, h, s, d]
    xs_all = (xh.reshape(B * NH, NT, 128, HD)
                .transpose(0, 2, 1, 3)
                .reshape(B * NH, 128, NT * HD)
                .astype(np.float32))

    cosT, sinT = _rope_tables()

    def w_full(W):
        ws = (W.T * ln_gamma[:, None]).astype(np.float32)   # [d, e]
        bias = (W @ ln_beta).astype(np.float32)             # [e]
        return np.concatenate([ws, bias[None, :]], axis=0)  # [65, e]

    wq_f = w_full(Wq)
    wk_f = w_full(Wk)
    wv_f = w_full(Wv)

    consts_base = np.zeros((128, CW), dtype=np.float32)
    consts_base[0:64, OFF_COS:OFF_COS + S] = cosT
    consts_base[0:64, OFF_SIN:OFF_SIN + S] = sinT
    consts_base[:, OFF_ID:OFF_ID + 128] = np.eye(128, dtype=np.float32)
    consts_base[:, OFF_ONES:OFF_ONES + S] = 1.0
    wblock = np.concatenate(
        [wq_f, _rot_cols(wq_f), wk_f, _rot_cols(wk_f), wv_f], axis=1)
    consts_base[0:65, OFF_W:OFF_W + 5 * HD] = wblock

    kbias = np.where(attention_mask, 0.0, MASK_NEG).astype(np.float32)

    in_maps = []
    for c in range(NCORES):
        consts_c = consts_base.copy()
        b = (c * UPC) // NH
        consts_c[64, OFF_KBIAS:OFF_KBIAS + S] = kbias[b]
        in_maps.append({
            "xs": xs_all[c * UPC:(c + 1) * UPC],
            "consts": consts_c,
        })

    res = run_bass_kernel_spmd(nc, in_maps, list(range(NCORES)))
    _CACHE["last_result"] = res

    probs = np.concatenate(
        [res.results[c]["probs_o"] for c in range(NCORES)], axis=0)
    probs = probs.reshape(B, NH, S, S)
    out = np.concatenate(
        [res.results[c]["out_o"] for c in range(NCORES)], axis=0)
    out = (out.reshape(B, NH, S, HD)
              .transpose(0, 2, 1, 3)
              .reshape(B, S, HID)
              .astype(np.float32))
    return out, probs
